# revision 1
# baseline (speedup 1.0000x reference)
"""GPT forward pass on 8 Trainium2 NeuronCores.

Sharding: token-parallel trunk. Core c owns q-tile c (rows 128c..128c+127)
of each of the 4 sequences (512 tokens/core). Attention needs all K/V, which
is AllGathered (bf16) across the 8 cores once per layer. The lm_head is
vocab-sharded (4000 cols/core) over an AllGather of the final hidden states.
All matmuls run in bf16 with fp32 PSUM accumulation; layernorm/softmax
statistics and residual stream stay fp32.

Softmax is computed in transposed layout: S^T[k,q] = (K^T).T @ Q^T, exp on
ScalarE, multiplicative causal mask on VectorE, and the denominators come for
free from the P@V matmul by appending a ones-column to V.
"""

import os
import sys

for _p in ("/opt/trn_rl_repo",):
    if os.path.isdir(_p) and _p not in sys.path:
        sys.path.insert(0, _p)

import numpy as np
import ml_dtypes

BF16NP = ml_dtypes.bfloat16

import concourse.bass as bass
import concourse.mybir as mybir
import concourse.tile as tile
from concourse import bacc
from concourse.bass_utils import run_bass_kernel_spmd
from concourse.masks import make_identity

F32 = mybir.dt.float32
BF = mybir.dt.bfloat16
AF = mybir.ActivationFunctionType

V, C, T, H, L, B = 32000, 1024, 1024, 16, 4, 4
HD = C // H          # 64
FF = 4 * C           # 4096
NCORES = 8
TL = 512             # local tokens per core (4 seqs x 128)
SEQ = B              # 4
NT = TL // 128       # 4  local t-tiles; tile tt holds seq tt rows
NCT = C // 128       # 8  c-tiles
NFT = FF // 128      # 32 f-tiles
VSH = V // NCORES    # 4000 vocab shard
NVC = 8
VCW = VSH // NVC     # 500
LN_EPS = 1e-5

KV_K = C * TL        # elems in K^T region of kv_loc
KV_SZ = 2 * C * TL   # elems per-core kv payload (K^T + V)

_prog_cache = {}


def _ap(t, offset, pattern):
    return bass.AP(tensor=t.tensor if isinstance(t, bass.AP) else t, offset=offset, ap=pattern)


def _build(LL=L, debug=False, sim=False):
    key = (LL, debug, sim)
    if key in _prog_cache:
        return _prog_cache[key]

    nc = bacc.Bacc("TRN2", target_bir_lowering=False, debug=False, num_devices=NCORES)

    x0 = nc.dram_tensor("x0", [TL, C], F32, kind="ExternalInput")
    maskT_d = nc.dram_tensor("maskT", [128, NCT, 128], BF, kind="ExternalInput")
    wq_d = nc.dram_tensor("wq", [L, C, C], BF, kind="ExternalInput")
    wk_d = nc.dram_tensor("wk", [L, C, C], BF, kind="ExternalInput")
    wv_d = nc.dram_tensor("wv", [L, C, C], BF, kind="ExternalInput")
    wo_d = nc.dram_tensor("wo", [L, C, C], BF, kind="ExternalInput")
    w1_d = nc.dram_tensor("w1", [L, C, FF], BF, kind="ExternalInput")
    w2_d = nc.dram_tensor("w2", [L, FF, C], BF, kind="ExternalInput")
    b1_d = nc.dram_tensor("b1", [L, FF], F32, kind="ExternalInput")
    bo_d = nc.dram_tensor("bo", [L, C], F32, kind="ExternalInput")
    b2_d = nc.dram_tensor("b2", [L, C], F32, kind="ExternalInput")
    ln1g_d = nc.dram_tensor("ln1g", [L, C], F32, kind="ExternalInput")
    ln1b_d = nc.dram_tensor("ln1b", [L, C], F32, kind="ExternalInput")
    ln2g_d = nc.dram_tensor("ln2g", [L, C], F32, kind="ExternalInput")
    ln2b_d = nc.dram_tensor("ln2b", [L, C], F32, kind="ExternalInput")
    lnfg_d = nc.dram_tensor("lnfg", [C], F32, kind="ExternalInput")
    lnfb_d = nc.dram_tensor("lnfb", [C], F32, kind="ExternalInput")
    wlm_d = nc.dram_tensor("wlm", [C, VSH], BF, kind="ExternalInput")
    blm_d = nc.dram_tensor("blm", [VSH], BF, kind="ExternalInput")

    logits_d = nc.dram_tensor("logits", [NCORES * TL, VSH], F32, kind="ExternalOutput")
    dbg_d = None
    if debug:
        dbg_d = nc.dram_tensor("dbg", [LL, TL, C], F32, kind="ExternalOutput")

    with tile.TileContext(nc) as tc:
        import contextlib

        with contextlib.ExitStack() as ctx:
            # SBUF pools (per-partition KB in comments)
            const = ctx.enter_context(tc.tile_pool(name="const", bufs=1))      # ~2.5
            xpool = ctx.enter_context(tc.tile_pool(name="x", bufs=1))          # 16
            hpool = ctx.enter_context(tc.tile_pool(name="h", bufs=5))          # 10
            tpool = ctx.enter_context(tc.tile_pool(name="hT", bufs=1))         # 8
            qtpool = ctx.enter_context(tc.tile_pool(name="qt", bufs=1))        # 8
            kvpool = ctx.enter_context(tc.tile_pool(name="kv", bufs=4))        # 4
            otpool = ctx.enter_context(tc.tile_pool(name="oT", bufs=1))        # 8
            big = ctx.enter_context(tc.tile_pool(name="big", bufs=2))          # 64
            wpool = ctx.enter_context(tc.tile_pool(name="w", bufs=6))          # 6
            gbpool = ctx.enter_context(tc.tile_pool(name="gb", bufs=1))        # 16
            misc = ctx.enter_context(tc.tile_pool(name="misc", bufs=2))        # ~1
            lntmp = ctx.enter_context(tc.tile_pool(name="lntmp", bufs=2))      # 8
            kts_pool = ctx.enter_context(tc.tile_pool(name="kts", bufs=3))     # 4
            pt_pool = ctx.enter_context(tc.tile_pool(name="pt", bufs=3))       # 6
            oraw_pool = ctx.enter_context(tc.tile_pool(name="oraw", bufs=2))   # 20
            rcp_pool = ctx.enter_context(tc.tile_pool(name="rcp", bufs=1))     # 8
            rb_pool = ctx.enter_context(tc.tile_pool(name="rb", bufs=1))       # 16
            lmh_pool = ctx.enter_context(tc.tile_pool(name="lmh", bufs=1))     # 8
            lgout = ctx.enter_context(tc.tile_pool(name="lgout", bufs=2))      # 4
            lgb_pool = ctx.enter_context(tc.tile_pool(name="lgb", bufs=1))     # 8
            ps_acc = ctx.enter_context(tc.tile_pool(name="psacc", bufs=5, space="PSUM"))
            ps_st = ctx.enter_context(tc.tile_pool(name="psst", bufs=2, space="PSUM"))
            ps_ov = ctx.enter_context(tc.tile_pool(name="psov", bufs=1, space="PSUM"))
            dram = ctx.enter_context(tc.tile_pool(name="dram", bufs=1, space="DRAM"))

            ident = const.tile([128, 128], BF, name="ident")
            make_identity(nc, ident)
            eps_t = const.tile([128, 1], F32, name="eps")
            nc.vector.memset(eps_t[:], LN_EPS)
            maskT = const.tile([128, NCT, 128], BF, name="maskT")
            nc.sync.dma_start(out=maskT[:], in_=maskT_d[:])

            kv_loc = dram.tile([KV_SZ], BF, name="kv_loc")
            hfT_loc = dram.tile([C * TL], BF, name="hfT_loc")
            hfT_full = dram.tile([NCORES * C * TL], BF, addr_space="Local" if sim else "Shared", name="hfT_full")
            rc_bounce = dram.tile([SEQ, H * 128], F32, name="rc_bounce")

            # persistent residual stream, fp32: tile tt = seq tt, partition j
            x_t = [xpool.tile([128, C], F32, tag=f"x{tt}", name=f"x{tt}") for tt in range(NT)]
            for tt in range(NT):
                nc.sync.dma_start(out=x_t[tt][:], in_=x0[tt * 128:(tt + 1) * 128, :])

            def bcast_row(dst, src_1d_tensor, offset, n):
                # replicate a [n] dram row across all partitions of dst [P, n]
                src = _ap(src_1d_tensor, offset, [[0, dst.shape[0]], [1, n]])
                nc.gpsimd.dma_start(out=dst[:], in_=src)

            def emit_ln(g_t, b_t):
                """LayerNorm over free dim of x_t -> transposed bf16 hT tiles."""
                h_tiles = []
                for tt in range(NT):
                    stats = misc.tile([128, 2, 6], F32, name="stats", tag="stats")
                    xv = x_t[tt][:].rearrange("p (s d) -> p s d", s=2)
                    nc.vector.bn_stats(out=stats[:, 0, :], in_=xv[:, 0, :])
                    nc.vector.bn_stats(out=stats[:, 1, :], in_=xv[:, 1, :])
                    mv = misc.tile([128, 2], F32, name="mv", tag="mv")
                    nc.vector.bn_aggr(out=mv[:], in_=stats[:])
                    rstd = misc.tile([128, 1], F32, name="rstd", tag="rstd")
                    nc.scalar.activation(rstd[:], mv[:, 1:2], AF.Sqrt, bias=eps_t[:])
                    nc.vector.reciprocal(rstd[:], rstd[:])
                    xn = lntmp.tile([128, C], F32, tag="xn", name="xn")
                    nc.vector.tensor_scalar(
                        out=xn[:], in0=x_t[tt][:], scalar1=mv[:, 0:1], scalar2=rstd[:],
                        op0=mybir.AluOpType.subtract, op1=mybir.AluOpType.mult,
                    )
                    nc.vector.tensor_mul(out=xn[:], in0=xn[:], in1=g_t[:])
                    h = hpool.tile([128, C], BF, tag="h", name="h")
                    nc.vector.tensor_add(out=h[:], in0=xn[:], in1=b_t[:])
                    h_tiles.append(h)
                hT_tiles = []
                for ct in range(NCT):
                    pst = ps_st.tile([128, 512], BF, tag="st", name="pst")
                    for tt in range(NT):
                        nc.tensor.transpose(
                            pst[:, tt * 128:(tt + 1) * 128],
                            h_tiles[tt][:, ct * 128:(ct + 1) * 128],
                            ident[:],
                        )
                    hT = tpool.tile([128, 512], BF, tag=f"hT{ct}", name=f"hT{ct}")
                    nc.vector.tensor_copy(out=hT[:], in_=pst[:])
                    hT_tiles.append(hT)
                return hT_tiles

            def load_w_tile(wd, l_idx, r0, c0, rows=128, cols=512):
                wt = wpool.tile([rows, cols], BF, tag="w", name="wt")
                nc.sync.dma_start(out=wt[:], in_=wd[l_idx, r0:r0 + rows, c0:c0 + cols])
                return wt

            for l in range(LL):
                lw = l % L
                g1 = gbpool.tile([128, C], F32, tag="g", name="g1")
                bcast_row(g1, ln1g_d, lw * C, C)
                bb1 = gbpool.tile([128, C], F32, tag="b", name="bb1")
                bcast_row(bb1, ln1b_d, lw * C, C)
                hT = emit_ln(g1, bb1)

                # ---- QKV projections ----
                # Q^T (resident), K^T (streamed to kv_loc): out[d,t] = sum_c W[c,d] hT[c,t]
                qT = []
                for name, wd in (("q", wq_d), ("k", wk_d)):
                    for dtg in range(2):
                        pss = [ps_acc.tile([128, 512], F32, tag="acc", name="acc") for _ in range(4)]
                        for ct in range(NCT):
                            wt = load_w_tile(wd, lw, ct * 128, dtg * 512)
                            for d4 in range(4):
                                nc.tensor.matmul(
                                    pss[d4][:], wt[:, d4 * 128:(d4 + 1) * 128], hT[ct][:],
                                    start=(ct == 0), stop=(ct == NCT - 1),
                                )
                        for d4 in range(4):
                            dt = dtg * 4 + d4
                            if name == "q":
                                ot = qtpool.tile([128, 512], BF, tag=f"qT{dt}", name=f"qT{dt}")
                                nc.vector.tensor_copy(out=ot[:], in_=pss[d4][:])
                                qT.append(ot)
                            else:
                                ot = kvpool.tile([128, 512], BF, tag="kv", name="kTs_out")
                                nc.vector.tensor_copy(out=ot[:], in_=pss[d4][:])
                                nc.sync.dma_start(
                                    out=_ap(kv_loc, dt * 128 * TL, [[TL, 128], [1, TL]]),
                                    in_=ot[:],
                                )
                # V natural (streamed in halves): out[t,c'] = sum_c hT[c,t] Wv[c,c']
                for nf in range(2):
                    pss = [ps_acc.tile([128, 512], F32, tag="acc", name="acc") for _ in range(4)]
                    for ct in range(NCT):
                        wt = load_w_tile(wv_d, lw, ct * 128, nf * 512)
                        for tt in range(NT):
                            nc.tensor.matmul(
                                pss[tt][:], hT[ct][:, tt * 128:(tt + 1) * 128], wt[:],
                                start=(ct == 0), stop=(ct == NCT - 1),
                            )
                    for tt in range(NT):
                        vt = kvpool.tile([128, 512], BF, tag="kv", name="v_out")
                        nc.vector.tensor_copy(out=vt[:], in_=pss[tt][:])
                        nc.sync.dma_start(
                            out=_ap(kv_loc, KV_K + tt * 128 * C + nf * 512, [[C, 128], [1, 512]]),
                            in_=vt[:],
                        )
                kv_full = dram.tile([NCORES * KV_SZ], BF, addr_space="Local" if sim else "Shared", name=f"kv_full{l}")
                if sim:
                    nc.sync.dma_start(
                        out=_ap(kv_full, 0, [[2048, KV_SZ // 2048], [1, 2048]]),
                        in_=_ap(kv_loc, 0, [[2048, KV_SZ // 2048], [1, 2048]]),
                    )
                else:
                    nc.gpsimd.collective_compute(
                        "AllGather",
                        mybir.AluOpType.bypass,
                        replica_groups=[list(range(NCORES))],
                        ins=[_ap(kv_loc, 0, [[2048, KV_SZ // 2048], [1, 2048]])],
                        outs=[_ap(kv_full, 0, [[2048, NCORES * KV_SZ // 2048], [1, 2048]])],
                    )

                # ---- attention ----
                # O^T as one tile: row c = ct*128 + p, free = (ct, t)
                oT = otpool.tile([128, NCT, 512], BF, tag="oT", name="oT")
                for s in range(SEQ):
                    # V for seq s, all ranks/heads, with a ones column per head:
                    # v_s[j, r, h, 0:64] = V_r[s*128+j, h*64+d]; v_s[..., 64] = 1
                    v_s = big.tile([128, NCT, H, HD + 1], BF, tag="big", name="vs")
                    nc.vector.memset(v_s[:, :, :, HD:HD + 1], 1.0)
                    for r in range(NCORES):
                        nc.sync.dma_start(
                            out=v_s[:, r, :, 0:HD],
                            in_=_ap(
                                kv_full,
                                r * KV_SZ + KV_K + s * 128 * C,
                                [[C, 128], [HD, H], [1, HD]],
                            ),
                        )
                    oraw = oraw_pool.tile([HD + 1, H, 128], F32, tag="oraw", name="oraw")
                    for h in range(H):
                        poff = (h % 2) * HD  # parity offset matches qT slices
                        kTs = kts_pool.tile([128, NCT, 128], BF, tag="kts", name="kts")
                        nc.sync.dma_start(
                            out=kTs[poff:poff + HD, :, :],
                            in_=_ap(
                                kv_full,
                                h * HD * TL + s * 128,
                                [[TL, HD], [KV_SZ, NCORES], [1, 128]],
                            ),
                        )
                        q_sl = qT[h // 2][poff:poff + HD, s * 128:(s + 1) * 128]
                        pT = pt_pool.tile([128, NCT, 128], BF, tag="pt", name="pt")
                        for half in range(2):
                            st = ps_st.tile([128, 4, 128], F32, tag="st", name="st")
                            for k4 in range(4):
                                nc.tensor.matmul(
                                    st[:, k4, :], kTs[poff:poff + HD, half * 4 + k4, :], q_sl,
                                    start=True, stop=True,
                                )
                            nc.scalar.activation(
                                pT[:, half * 4:half * 4 + 4, :], st[:], AF.Exp
                            )
                        nc.vector.tensor_mul(out=pT[:], in0=pT[:], in1=maskT[:])
                        ov = ps_ov.tile([128, 128], F32, tag="ov", name="ov")
                        for kt in range(NCT):
                            nc.tensor.matmul(
                                ov[0:HD + 1, :], v_s[:, kt, h, :], pT[:, kt, :],
                                start=(kt == 0), stop=(kt == NCT - 1),
                            )
                        nc.vector.tensor_copy(out=oraw[:, h, :], in_=ov[0:HD + 1, :])
                    # denominators -> reciprocal -> broadcast over 64 partitions
                    recips = rcp_pool.tile([1, H, 128], F32, tag="recips", name="recips")
                    nc.vector.reciprocal(recips[:], oraw[HD:HD + 1, :, :])
                    nc.sync.dma_start(out=rc_bounce[s, :], in_=recips[:])
                    rb = rb_pool.tile([HD, H, 128], F32, tag="rb", name="rb")
                    nc.gpsimd.dma_start(
                        out=rb[:], in_=_ap(rc_bounce, s * H * 128, [[0, HD], [128, H], [1, 128]])
                    )
                    # even heads: normalize straight into oT (partitions 0-63);
                    # odd heads: stage then DMA into partitions 64-127
                    oS = oraw_pool.tile([HD, NCT, 128], BF, tag="oS", name="oS")
                    for h in range(H):
                        if h % 2 == 0:
                            dst = oT[0:HD, h // 2, s * 128:(s + 1) * 128]
                        else:
                            dst = oS[:, h // 2, :]
                        nc.vector.tensor_mul(out=dst, in0=oraw[0:HD, h, :], in1=rb[:, h, :])
                    nc.sync.dma_start(
                        out=oT[HD:128, :, s * 128:(s + 1) * 128], in_=oS[:],
                    )

                # ---- output projection + residual ----
                bo_t = gbpool.tile([128, C], F32, tag="b", name="bo_t")
                bcast_row(bo_t, bo_d, lw * C, C)
                for nf in range(2):
                    pss = [ps_acc.tile([128, 512], F32, tag="acc", name="acc") for _ in range(4)]
                    for ct in range(NCT):
                        wt = load_w_tile(wo_d, lw, ct * 128, nf * 512)
                        for tt in range(NT):
                            nc.tensor.matmul(
                                pss[tt][:], oT[:, ct, tt * 128:(tt + 1) * 128], wt[:],
                                start=(ct == 0), stop=(ct == NCT - 1),
                            )
                    for tt in range(NT):
                        xs = x_t[tt][:, nf * 512:(nf + 1) * 512]
                        nc.vector.tensor_add(out=xs, in0=xs, in1=pss[tt][:])
                        nc.vector.tensor_add(out=xs, in0=xs, in1=bo_t[:, nf * 512:(nf + 1) * 512])

                # ---- FFN ----
                g2 = gbpool.tile([128, C], F32, tag="g", name="g2")
                bcast_row(g2, ln2g_d, lw * C, C)
                bb2 = gbpool.tile([128, C], F32, tag="b", name="bb2")
                bcast_row(bb2, ln2b_d, lw * C, C)
                h2T = emit_ln(g2, bb2)

                b1_t = misc.tile([128, NFT], F32, tag="b1", name="b1_t")
                nc.gpsimd.dma_start(
                    out=b1_t[:], in_=_ap(b1_d, lw * FF, [[1, 128], [128, NFT]])
                )
                ug = big.tile([128, NFT, 512], BF, tag="big", name="ug")
                for fg in range(8):
                    pss = [ps_acc.tile([128, 512], F32, tag="acc", name="acc") for _ in range(4)]
                    for ct in range(NCT):
                        wt = load_w_tile(w1_d, lw, ct * 128, fg * 512)
                        for f4 in range(4):
                            nc.tensor.matmul(
                                pss[f4][:], wt[:, f4 * 128:(f4 + 1) * 128], h2T[ct][:],
                                start=(ct == 0), stop=(ct == NCT - 1),
                            )
                    for f4 in range(4):
                        ft = fg * 4 + f4
                        nc.scalar.activation(
                            ug[:, ft, :], pss[f4][:], AF.Gelu, bias=b1_t[:, ft:ft + 1]
                        )

                b2_t = gbpool.tile([128, C], F32, tag="b", name="b2_t")
                bcast_row(b2_t, b2_d, lw * C, C)
                for nf in range(2):
                    pss = [ps_acc.tile([128, 512], F32, tag="acc", name="acc") for _ in range(4)]
                    for ft in range(NFT):
                        wt = load_w_tile(w2_d, lw, ft * 128, nf * 512)
                        for tt in range(NT):
                            nc.tensor.matmul(
                                pss[tt][:], ug[:, ft, tt * 128:(tt + 1) * 128], wt[:],
                                start=(ft == 0), stop=(ft == NFT - 1),
                            )
                    for tt in range(NT):
                        xs = x_t[tt][:, nf * 512:(nf + 1) * 512]
                        nc.vector.tensor_add(out=xs, in0=xs, in1=pss[tt][:])
                        nc.vector.tensor_add(out=xs, in0=xs, in1=b2_t[:, nf * 512:(nf + 1) * 512])

                if debug:
                    for tt in range(NT):
                        nc.sync.dma_start(
                            out=dbg_d[l, tt * 128:(tt + 1) * 128, :], in_=x_t[tt][:]
                        )

            # ---- final LN, AllGather h_f^T, lm_head ----
            gf = gbpool.tile([128, C], F32, tag="g", name="gf")
            bcast_row(gf, lnfg_d, 0, C)
            bft = gbpool.tile([128, C], F32, tag="b", name="bft")
            bcast_row(bft, lnfb_d, 0, C)
            hfT = emit_ln(gf, bft)
            for ct in range(NCT):
                nc.sync.dma_start(
                    out=_ap(hfT_loc, ct * 128 * TL, [[TL, 128], [1, TL]]),
                    in_=hfT[ct][:],
                )
            if sim:
                nc.sync.dma_start(
                    out=_ap(hfT_full, 0, [[2048, C * TL // 2048], [1, 2048]]),
                    in_=_ap(hfT_loc, 0, [[2048, C * TL // 2048], [1, 2048]]),
                )
            else:
                nc.gpsimd.collective_compute(
                    "AllGather",
                    mybir.AluOpType.bypass,
                    replica_groups=[list(range(NCORES))],
                    ins=[_ap(hfT_loc, 0, [[2048, C * TL // 2048], [1, 2048]])],
                    outs=[_ap(hfT_full, 0, [[2048, NCORES * C * TL // 2048], [1, 2048]])],
                )

            # lm_head in vocab halves: wlm half resident, hf streamed per r
            for vq in range(2):
                wlm_q = big.tile([128, NCT, 4 * VCW], BF, tag="big", name="wlmq")
                for ct in range(NCT):
                    nc.sync.dma_start(
                        out=wlm_q[:, ct, :],
                        in_=wlm_d[ct * 128:(ct + 1) * 128, vq * 4 * VCW:(vq + 1) * 4 * VCW],
                    )
                blm_qs = []
                for v4 in range(4):
                    bq = lgb_pool.tile([128, VCW], BF, tag="lgb", name="blmq", bufs=4)
                    bcast_row(bq, blm_d, (vq * 4 + v4) * VCW, VCW)
                    blm_qs.append(bq)
                for r in range(NCORES):
                    hfr = lmh_pool.tile([128, NCT, 512], BF, tag="hfr", name="hfr")
                    for ct in range(NCT):
                        nc.sync.dma_start(
                            out=hfr[:, ct, :],
                            in_=_ap(hfT_full, r * C * TL + ct * 128 * TL, [[TL, 128], [1, TL]]),
                        )
                    for ts in range(NT):
                        pss = [ps_acc.tile([128, VCW], F32, tag="acc", name="acc") for _ in range(4)]
                        for ct in range(NCT):
                            for v4 in range(4):
                                nc.tensor.matmul(
                                    pss[v4][:],
                                    hfr[:, ct, ts * 128:(ts + 1) * 128],
                                    wlm_q[:, ct, v4 * VCW:(v4 + 1) * VCW],
                                    start=(ct == 0), stop=(ct == NCT - 1),
                                )
                        for v4 in range(4):
                            vc = vq * 4 + v4
                            lg = lgout.tile([128, VCW], F32, tag="lg", name="lg")
                            nc.vector.tensor_add(
                                out=lg[:], in0=pss[v4][:],
                                in1=blm_qs[v4][:],
                            )
                            row0 = r * TL + ts * 128
                            nc.sync.dma_start(
                                out=logits_d[row0:row0 + 128, vc * VCW:(vc + 1) * VCW],
                                in_=lg[:],
                            )

    nc.compile()
    _prog_cache[key] = nc
    return nc


def _prep_inputs(inputs):
    f = {k: np.asarray(v) for k, v in inputs.items()}
    idx = f["idx"].astype(np.int64)
    emb = f["emb"].astype(np.float32)
    pos = f["pos_enc"].astype(np.float32)
    x_full = emb[idx] + pos[None, :, :]          # [B, T, C] f32

    scale = HD ** -0.5
    bf = lambda a: np.ascontiguousarray(a, dtype=np.float32).astype(BF16NP)
    shared = {
        "wq": bf(f["Wq"] * scale),
        "wk": bf(f["Wk"]),
        "wv": bf(f["Wv"]),
        "wo": bf(f["Wo"]),
        "w1": bf(f["W1"]),
        "w2": bf(f["W2"]),
        "b1": f["b1"].astype(np.float32),
        "bo": f["bo"].astype(np.float32),
        "b2": f["b2"].astype(np.float32),
        "ln1g": f["ln1_g"].astype(np.float32),
        "ln1b": f["ln1_b"].astype(np.float32),
        "ln2g": f["ln2_g"].astype(np.float32),
        "ln2b": f["ln2_b"].astype(np.float32),
        "lnfg": f["lnf_g"].astype(np.float32),
        "lnfb": f["lnf_b"].astype(np.float32),
    }
    wlm_f = f["Wlm"].astype(np.float32)
    blm_f = f["blm"].astype(np.float32)

    in_maps = []
    kk = np.arange(T)[:, None]
    for c in range(NCORES):
        x0_c = np.ascontiguousarray(
            x_full[:, 128 * c:128 * (c + 1), :].reshape(TL, C), dtype=np.float32
        )
        jj = np.arange(128)[None, :]
        m = (kk <= 128 * c + jj).astype(np.float32)      # [T, 128]
        maskT_c = np.ascontiguousarray(
            m.reshape(NCT, 128, 128).transpose(1, 0, 2)
        ).astype(BF16NP)                                  # [128(kk), 8(kt), 128(j)]
        im = dict(shared)
        im["x0"] = x0_c
        im["maskT"] = maskT_c
        im["wlm"] = np.ascontiguousarray(wlm_f[:, c * VSH:(c + 1) * VSH]).astype(BF16NP)
        im["blm"] = np.ascontiguousarray(blm_f[c * VSH:(c + 1) * VSH]).astype(BF16NP)
        in_maps.append(im)
    return in_maps


def kernel(**inputs):
    nc = _build()
    in_maps = _prep_inputs(inputs)
    res = run_bass_kernel_spmd(nc, in_maps, list(range(NCORES)))
    # per-core logits rows are [r(8), s(4), j(128)]; vocab sharded on cores
    parts = [r["logits"].reshape(NCORES, SEQ, 128, VSH) for r in res.results]
    full = np.concatenate(parts, axis=-1)                 # [r, s, j, V]
    full = full.transpose(1, 0, 2, 3).reshape(B, T, V)    # [s, r*128+j, V]
    return np.ascontiguousarray(full, dtype=np.float32)



# revision 16
# speedup vs baseline: 1.2403x; 1.2403x over previous
"""GPT forward pass on 8 Trainium2 NeuronCores — fp8 attention core + bf16
value/FFN/lm paths, head-parallel attention, two-stream software pipeline.

Sharding:
- Trunk (LN/QKV/Wo/FFN/lm_head) is token-parallel: core r owns q-tile r
  (rows 128r..128r+127) of each of the 4 sequences (512 tokens/core).
- Attention is head-parallel: core r owns heads {2r, 2r+1} for all 4 seqs,
  exchanged via AllToAll (Q/K/V out, O back). This makes causal skipping
  SPMD-uniform: unit attention computes only k-tiles kt <= qt and the
  diagonal mask sits at a static slot (one constant triangular multiply).
- The batch is split into two independent streams (seqs {0,1} / {2,3})
  interleaved through each layer so one stream's compute hides the other
  stream's AllToAll latency.
- lm_head is token-sharded: each core computes its 512 tokens against the
  full (padded) vocab, weights streamed, no collective. Logits exit fp16;
  blm is added on host.

Precision (validated against a numpy emulation of the exact quantization
chain; maxrel ~9e-3 vs the 2e-2 gate): Q/K projections and S = K^T Q run in
fp8e4m3 with DoubleRow perf mode (softmax damps their quantization noise).
The value path (V, P@V, Wo), FFN and lm_head run in bf16 — their errors hit
the residual stream and logits directly and fp8 there blows the error
budget. PSUM accumulation is fp32 everywhere; LN stats, softmax and the
residual stream stay fp32. The residual is carried at 256x (LN is
affine-invariant) so Wo/W2 PSUM deltas add in without a rescale pass.
Softmax denominators come free from a ones-column in V; biases enter as
rank-1 (K=1) matmul accumulations; b1 rides the Gelu activation bias.
"""

import os
import sys

for _p in ("/opt/trn_rl_repo",):
    if os.path.isdir(_p) and _p not in sys.path:
        sys.path.insert(0, _p)

import numpy as np
import ml_dtypes

FP8NP = ml_dtypes.float8_e4m3
BF16NP = ml_dtypes.bfloat16

import concourse.bass as bass
import concourse.mybir as mybir
import concourse.tile as tile
from concourse import bacc
from concourse.bass_utils import run_bass_kernel_spmd
from concourse.masks import make_identity

F32 = mybir.dt.float32
BF = mybir.dt.bfloat16
FP8 = mybir.dt.float8e4
FP16 = mybir.dt.float16
AF = mybir.ActivationFunctionType
DR = mybir.MatmulPerfMode.DoubleRow

V, C, T, H, L, B = 32000, 1024, 1024, 16, 4, 4
HD = C // H          # 64
FF = 4 * C           # 4096
NCORES = 8
TL = 512             # local tokens per core (4 seqs x 128)
SEQ = B              # 4
NT = TL // 128       # 4 local t-tiles; tile tt = seq tt
NCT = C // 128       # 8 c-tiles
LN_EPS = 1e-5

SW = 64.0            # fp8 weight prescale (wq, wk)
SQK = 4.0            # stored scale of q,k  (copy scale SQK/SW = 1/16)
RS = 256.0           # residual stream scale (wo, w2 pre-multiplied by RS)
EXPS = (HD ** -0.5) / (SQK * SQK)   # exp scale: 1/128

# Per-stream AllToAll slot layout (BYTES; buffers are fp8-typed, bf16
# payloads are written through AP.bitcast). Stream g carries seqs {2g,2g+1}.
QOFFH, KOFFH, VOFFH = 0, 32768, 65536
SLOTH = 131072       # qT[128,256]fp8 + kT[128,256]fp8 + v[256,128]bf16
OSLOTH = 65536       # o return slot: [4 units * 64 dims, 128 tokens] bf16

VCW = 512            # lm_head vocab chunk (1KB elem => full DMA bw)
VP = 32256           # vocab padded to 63*512 (host pads Wlm, slices logits)
NVC = VP // VCW      # 63

_prog_cache = {}


def _ap(t, offset, pattern):
    return bass.AP(tensor=t.tensor if isinstance(t, bass.AP) else t, offset=offset, ap=pattern)


def _build(LL=L, debug=False):
    key = (LL, debug)
    if key in _prog_cache:
        return _prog_cache[key]

    nc = bacc.Bacc("TRN2", target_bir_lowering=False, debug=False, num_devices=NCORES)

    x0 = nc.dram_tensor("x0", [TL, C], F32, kind="ExternalInput")
    tri_d = nc.dram_tensor("tri", [128, 128], BF, kind="ExternalInput")
    wq_d = nc.dram_tensor("wq", [L, C, C], FP8, kind="ExternalInput")
    wk_d = nc.dram_tensor("wk", [L, C, C], FP8, kind="ExternalInput")
    wv_d = nc.dram_tensor("wv", [L, C, C], BF, kind="ExternalInput")
    wo_d = nc.dram_tensor("wo", [L, C, C], BF, kind="ExternalInput")
    w1_d = nc.dram_tensor("w1", [L, C, FF], BF, kind="ExternalInput")
    w2_d = nc.dram_tensor("w2", [L, FF, C], BF, kind="ExternalInput")
    bo1_d = nc.dram_tensor("bo1", [L, C], BF, kind="ExternalInput")   # bo*256
    b21_d = nc.dram_tensor("b21", [L, C], BF, kind="ExternalInput")   # b2*256
    # packed per-channel columns (partition-major, see _prep_inputs):
    # [ln1g(L*8) | ln1b | ln2g | ln2b | lnfg(8) | lnfb(8) | b1(L*32)]
    colc_d = nc.dram_tensor("colc", [128, 272], F32, kind="ExternalInput")
    wlm_d = nc.dram_tensor("wlm", [C, VP], BF, kind="ExternalInput")

    logits_d = nc.dram_tensor("logits", [TL, VP], FP16, kind="ExternalOutput")
    dbg_d = None
    if debug:
        dbg_d = nc.dram_tensor("dbg", [LL, TL, C], F32, kind="ExternalOutput")

    with tile.TileContext(nc) as tc:
        import contextlib

        with contextlib.ExitStack() as ctx:
            # SBUF pools (approx per-partition bytes in comments)
            const = ctx.enter_context(tc.tile_pool(name="const", bufs=1))     # ~3K
            xpool = ctx.enter_context(tc.tile_pool(name="x", bufs=1))         # 16K
            xnpool = ctx.enter_context(tc.tile_pool(name="xn", bufs=2))       # 4K
            htpool = ctx.enter_context(tc.tile_pool(name="ht", bufs=1))       # 28K
            wp8 = ctx.enter_context(tc.tile_pool(name="wp8", bufs=4))         # 16K
            wpb = ctx.enter_context(tc.tile_pool(name="wpb", bufs=4))         # 32K
            stg = ctx.enter_context(tc.tile_pool(name="stg", bufs=4))         # 8K
            kqpool = ctx.enter_context(tc.tile_pool(name="kq", bufs=4))       # 8K
            vpool = ctx.enter_context(tc.tile_pool(name="vu", bufs=2))        # 2.5K
            ptpool = ctx.enter_context(tc.tile_pool(name="pt", bufs=3))       # 6K
            otu_pool = ctx.enter_context(tc.tile_pool(name="otu", bufs=2))    # 4K
            otpool = ctx.enter_context(tc.tile_pool(name="ot", bufs=1))       # 8K
            dnpool = ctx.enter_context(tc.tile_pool(name="dn", bufs=2))       # ~1K
            rbpool = ctx.enter_context(tc.tile_pool(name="rb", bufs=2))       # 4K
            gbpool = ctx.enter_context(tc.tile_pool(name="gb", bufs=2))       # 8K
            ugpool = ctx.enter_context(tc.tile_pool(name="ug", bufs=1))       # 32K
            lmw = ctx.enter_context(tc.tile_pool(name="lmw", bufs=2))         # 16K
            lgpool = ctx.enter_context(tc.tile_pool(name="lg", bufs=2))       # 8K
            misc = ctx.enter_context(tc.tile_pool(name="misc", bufs=2))       # ~1K
            psA = ctx.enter_context(tc.tile_pool(name="psA", bufs=6, space="PSUM"))
            ovp = ctx.enter_context(tc.tile_pool(name="ovp", bufs=2, space="PSUM"))
            dram = ctx.enter_context(tc.tile_pool(name="dram", bufs=1, space="DRAM"))

            ident = const.tile([128, 128], BF, name="ident")
            make_identity(nc, ident)
            eps_t = const.tile([128, 1], F32, name="eps")
            nc.vector.memset(eps_t[:], LN_EPS)
            tri = const.tile([128, 128], BF, name="tri")
            nc.gpsimd.dma_start(out=tri[:], in_=tri_d[:])
            ones1 = const.tile([1, 128], BF, name="ones1")
            nc.vector.memset(ones1[:], 1.0)

            a2a_in = [dram.tile([NCORES * SLOTH], FP8, name=f"a2a_in{g}") for g in range(2)]
            a2a_out = [dram.tile([NCORES * SLOTH], FP8, name=f"a2a_out{g}") for g in range(2)]
            o2a_in = [dram.tile([NCORES * OSLOTH], FP8, name=f"o2a_in{g}") for g in range(2)]
            o2a_out = [dram.tile([NCORES * OSLOTH], FP8, name=f"o2a_out{g}") for g in range(2)]

            # persistent residual stream fp32 (carried at 256x): tile tt = seq tt
            x_t = [xpool.tile([128, C], F32, tag=f"x{tt}", name=f"x{tt}") for tt in range(NT)]
            for tt in range(NT):
                nc.sync.dma_start(out=x_t[tt][:], in_=x0[tt * 128:(tt + 1) * 128, :])

            colc = const.tile([128, 272], F32, name="colc")
            nc.gpsimd.dma_start(out=colc[:], in_=colc_d[:])

            def a2a(tin, tout, nbytes):
                nc.gpsimd.collective_compute(
                    "AllToAll",
                    mybir.AluOpType.bypass,
                    replica_groups=[list(range(NCORES))],
                    ins=[_ap(tin, 0, [[1, nbytes]])],
                    outs=[_ap(tout, 0, [[1, nbytes]])],
                )

            def emit_ln(g, goff, boff, tag, fp8_also=False):
                """LN over free dim of stream g's x tiles -> hT [128, NCT, 256]
                bf16 (and optionally an fp8 copy for the Q/K DoubleRow path)."""
                xns = []
                for tl in range(2):
                    xt = x_t[2 * g + tl]
                    stats = misc.tile([128, 2, 6], F32, name="stats", tag="stats")
                    xv = xt[:].rearrange("p (s d) -> p s d", s=2)
                    nc.vector.bn_stats(out=stats[:, 0, :], in_=xv[:, 0, :])
                    nc.vector.bn_stats(out=stats[:, 1, :], in_=xv[:, 1, :])
                    mv = misc.tile([128, 2], F32, name="mv", tag="mv")
                    nc.vector.bn_aggr(out=mv[:], in_=stats[:])
                    rstd = misc.tile([128, 1], F32, name="rstd", tag="rstd")
                    nc.scalar.activation(rstd[:], mv[:, 1:2], AF.Sqrt, bias=eps_t[:])
                    nc.vector.reciprocal(rstd[:], rstd[:])
                    xn = xnpool.tile([128, C], BF, tag="xn", name="xn")
                    nc.vector.tensor_scalar(
                        out=xn[:], in0=xt[:], scalar1=mv[:, 0:1], scalar2=rstd[:],
                        op0=mybir.AluOpType.subtract, op1=mybir.AluOpType.mult,
                    )
                    xns.append(xn)
                hTb = htpool.tile([128, NCT, 256], BF, tag=f"{tag}{g}", name=f"{tag}{g}")
                hT8 = None
                if fp8_also:
                    hT8 = htpool.tile([128, NCT, 256], FP8, tag=f"{tag}8{g}", name=f"{tag}8{g}")
                for ct in range(NCT):
                    pst = psA.tile([128, 256], BF, tag="ps", name="pst")
                    for tl in range(2):
                        nc.tensor.transpose(
                            pst[:, tl * 128:(tl + 1) * 128],
                            xns[tl][:, ct * 128:(ct + 1) * 128],
                            ident[:],
                        )
                    # fused per-channel gain/bias (per-partition after transpose)
                    nc.vector.tensor_scalar(
                        out=hTb[:, ct, :], in0=pst[:],
                        scalar1=colc[:, goff + ct:goff + ct + 1],
                        scalar2=colc[:, boff + ct:boff + ct + 1],
                        op0=mybir.AluOpType.mult, op1=mybir.AluOpType.add,
                    )
                    if fp8_also:
                        nc.gpsimd.tensor_copy(out=hT8[:, ct, :], in_=hTb[:, ct, :])
                return hTb, hT8

            def load_w8(wd, layer_off, col0):
                # [128, 8, 512] fp8: rows 0..1023, cols col0..col0+511
                wt = wp8.tile([128, 8, 512], FP8, tag="w8", name="w8t")
                nc.sync.dma_start(
                    out=wt[:],
                    in_=_ap(wd, layer_off + col0,
                            [[C, 128], [128 * C, 8], [1, 512]]),
                )
                return wt

            def load_wb(wd, layer_off, row0, col0, nrow_t, cols, rows_stride):
                # [128, nrow_t, cols] bf16 tile
                wt = wpb.tile([128, nrow_t, cols], BF, tag="wb", name="wbt")
                nc.sync.dma_start(
                    out=wt[:],
                    in_=_ap(wd, layer_off + row0 * rows_stride + col0,
                            [[rows_stride, 128], [128 * rows_stride, nrow_t], [1, cols]]),
                )
                return wt

            def qkv(g, hTb, hT8, wq2, wk2, wv2):
                """Q/K transposed (fp8 DR) + V natural (bf16) -> a2a_in[g]."""
                for off, wts in ((QOFFH, wq2), (KOFFH, wk2)):
                    for half in range(2):
                        wt = wts[half]
                        pss = [psA.tile([128, 256], F32, tag="ps", name="acc") for _ in range(4)]
                        for j in range(4):
                            for d4 in range(4):
                                nc.tensor.matmul(
                                    pss[d4][:], wt[:, 2 * j:2 * j + 2, d4 * 128:(d4 + 1) * 128],
                                    hT8[:, 2 * j:2 * j + 2, :],
                                    start=(j == 0), stop=(j == 3), perf_mode=DR,
                                )
                        qs = stg.tile([128, 4, 256], FP8, tag="stg", name="qs")
                        for d4 in range(4):
                            nc.scalar.activation(qs[:, d4, :], pss[d4][:], AF.Copy, scale=SQK / SW)
                        nc.gpsimd.dma_start(
                            out=_ap(a2a_in[g], (half * 4) * SLOTH + off,
                                    [[256, 128], [SLOTH, 4], [1, 256]]),
                            in_=qs[:],
                        )
                for nf in range(2):
                    wt = wv2[nf]
                    pss = [psA.tile([128, 512], F32, tag="ps", name="acc") for _ in range(2)]
                    for ct in range(NCT):
                        for tl in range(2):
                            nc.tensor.matmul(
                                pss[tl][:], hTb[:, ct, tl * 128:(tl + 1) * 128],
                                wt[:, ct, :],
                                start=(ct == 0), stop=(ct == NCT - 1),
                            )
                    for tl in range(2):
                        vs = stg.tile([128, 512], BF, tag="stgv", name="vs")
                        nc.vector.tensor_copy(out=vs[:], in_=pss[tl][:])
                        # v region rows are (s_local*128+j), 256B/row (bf16)
                        nc.gpsimd.dma_start(
                            out=_ap(a2a_in[g], (nf * 4) * SLOTH + VOFFH + tl * 128 * 256,
                                    [[256, 128], [SLOTH, 4], [1, 256]]),
                            in_=vs[:].rearrange("p (dt c) -> p dt c", dt=4).bitcast(FP8),
                        )
                a2a(a2a_in[g], a2a_out[g], NCORES * SLOTH)

            def attention(g):
                """4 (seq,parity) units, full causal seq each; O -> o2a_in[g]."""
                for u in range(4):
                    sl, p = u % 2, u // 2
                    kT_u = kqpool.tile([32, 8, 2, 128], FP8, tag="kq", name="kT_u")
                    q_u = kqpool.tile([32, 8, 2, 128], FP8, tag="kq", name="q_u")
                    for two in range(2):
                        nc.sync.dma_start(
                            out=kT_u[:, :, two, :],
                            in_=_ap(a2a_out[g], KOFFH + p * 16384 + two * 8192 + sl * 128,
                                    [[256, 32], [SLOTH, 8], [1, 128]]),
                        )
                        nc.sync.dma_start(
                            out=q_u[:, :, two, :],
                            in_=_ap(a2a_out[g], QOFFH + p * 16384 + two * 8192 + sl * 128,
                                    [[256, 32], [SLOTH, 8], [1, 128]]),
                        )
                    v_u = vpool.tile([128, 8, 80], BF, tag="vu", name="v_u")
                    nc.sync.dma_start(
                        out=v_u[:, :, 0:64].bitcast(FP8),
                        in_=_ap(a2a_out[g], VOFFH + sl * 128 * 256 + p * 128,
                                [[256, 128], [SLOTH, 8], [1, 128]]),
                    )
                    nc.gpsimd.memset(v_u[:, :, 64:65], 1.0)

                    oTu = otu_pool.tile([64, 8, 128], BF, tag="otu", name="oTu")
                    for half in range(2):
                        ov = ovp.tile([128, 4, 128], F32, tag="ov", name="ov")
                        for qq in range(4):
                            qt = half * 4 + qq
                            n = qt + 1
                            pT = ptpool.tile([128, 8, 128], BF, tag="pt", name="pT")
                            for base in range(0, n, 4):
                                cnt = min(4, n - base)
                                st = psA.tile([128, 4, 128], F32, tag="ps", name="st")
                                for kk in range(cnt):
                                    nc.tensor.matmul(
                                        st[:, kk, :], kT_u[:, base + kk, :, :],
                                        q_u[:, qt, :, :],
                                        start=True, stop=True, perf_mode=DR,
                                    )
                                nc.scalar.activation(
                                    pT[:, base:base + cnt, :], st[:, 0:cnt, :],
                                    AF.Exp, scale=EXPS,
                                )
                            # causal mask: only the diagonal tile is partial
                            nc.vector.tensor_mul(
                                out=pT[:, qt, :], in0=pT[:, qt, :], in1=tri[:]
                            )
                            for kt in range(n):
                                nc.tensor.matmul(
                                    ov[0:65, qq, :], v_u[:, kt, 0:65], pT[:, kt, :],
                                    start=(kt == 0), stop=(kt == n - 1),
                                )
                        # denominators -> recip -> broadcast (TensorE) -> normalize
                        dnr = dnpool.tile([1, 4, 128], BF, tag="dn", name="dnr")
                        with nc.allow_low_precision(reason="softmax denom recip bf16"):
                            nc.vector.reciprocal(dnr[:], ov[64:65, :, :])
                        rbp = psA.tile([64, 4, 128], F32, tag="ps", name="rbp")
                        nc.tensor.matmul(rbp[:], ones1[0:1, 0:64], dnr[:],
                                         start=True, stop=True)
                        rbs = rbpool.tile([64, 4, 128], F32, tag="rb", name="rbs")
                        nc.scalar.activation(rbs[:], rbp[:], AF.Copy)
                        nc.vector.tensor_mul(
                            out=oTu[:, half * 4:half * 4 + 4, :],
                            in0=ov[0:64, :, :],
                            in1=rbs[:],
                        )
                    nc.gpsimd.dma_start(
                        out=_ap(o2a_in[g], (p * 2 + sl) * 16384,
                                [[256, 64], [OSLOTH, 8], [1, 256]]),
                        in_=oTu[:].bitcast(FP8),
                    )
                a2a(o2a_in[g], o2a_out[g], NCORES * OSLOTH)

            def wo_resid(g, lw, wo2, borow):
                oT = otpool.tile([128, NCT, 256], BF, tag=f"oT{g}", name=f"oT{g}")
                for uu in range(8):
                    for p in range(2):
                        nc.sync.dma_start(
                            out=oT[p * 64:(p + 1) * 64, uu, :].rearrange(
                                "p (s j) -> p s j", s=2).bitcast(FP8),
                            in_=_ap(o2a_out[g], uu * OSLOTH + p * 32768,
                                    [[256, 64], [16384, 2], [1, 256]]),
                        )
                for nf in range(2):
                    wt = wo2[nf]
                    pss = [psA.tile([128, 512], F32, tag="ps", name="acc") for _ in range(2)]
                    for ct in range(NCT):
                        for tl in range(2):
                            nc.tensor.matmul(
                                pss[tl][:], oT[:, ct, tl * 128:(tl + 1) * 128],
                                wt[:, ct, :],
                                start=(ct == 0), stop=False,
                            )
                    for tl in range(2):
                        nc.tensor.matmul(
                            pss[tl][:], ones1[0:1, :], borow[0:1, nf * 512:(nf + 1) * 512],
                            start=False, stop=True,
                        )
                        xs = x_t[2 * g + tl][:, nf * 512:(nf + 1) * 512]
                        nc.vector.tensor_add(out=xs, in0=xs, in1=pss[tl][:])

            for l in range(LL):
                lw = l % L
                # stream A: LN1 + QKV + A2A; stream B fills the gap
                hA, hA8 = emit_ln(0, lw * 8, 32 + lw * 8, "hT", fp8_also=True)
                wq2 = [load_w8(wq_d, lw * C * C, half * 512) for half in range(2)]
                wk2 = [load_w8(wk_d, lw * C * C, half * 512) for half in range(2)]
                wv2 = [load_wb(wv_d, lw * C * C, 0, nf * 512, 8, 512, C) for nf in range(2)]
                qkv(0, hA, hA8, wq2, wk2, wv2)
                hB, hB8 = emit_ln(1, lw * 8, 32 + lw * 8, "hT", fp8_also=True)
                qkv(1, hB, hB8, wq2, wk2, wv2)

                attention(0)
                attention(1)

                wo2 = [load_wb(wo_d, lw * C * C, 0, nf * 512, 8, 512, C) for nf in range(2)]
                borow = gbpool.tile([1, C], BF, tag="bo", name="borow")
                nc.gpsimd.dma_start(out=borow[:], in_=bo1_d[lw:lw + 1, :])
                wo_resid(0, lw, wo2, borow)
                h2A, _ = emit_ln(0, 64 + lw * 8, 96 + lw * 8, "h2T")
                wo_resid(1, lw, wo2, borow)
                h2B, _ = emit_ln(1, 64 + lw * 8, 96 + lw * 8, "h2T")

                # ---- FFN up: per-fg weight tiles shared by both streams ----
                ugs = [ugpool.tile([128, FF // 128, 256], BF, tag=f"ug{g}", name=f"ug{g}")
                       for g in range(2)]
                for fg in range(8):
                    w1t = load_wb(w1_d, lw * C * FF, 0, fg * 512, 8, 512, FF)
                    for g, h2T in ((0, h2A), (1, h2B)):
                        pss = [psA.tile([128, 256], F32, tag="ps", name="acc") for _ in range(4)]
                        for ct in range(NCT):
                            for f4 in range(4):
                                nc.tensor.matmul(
                                    pss[f4][:], w1t[:, ct, f4 * 128:(f4 + 1) * 128],
                                    h2T[:, ct, :],
                                    start=(ct == 0), stop=(ct == NCT - 1),
                                )
                        for f4 in range(4):
                            ft = fg * 4 + f4
                            nc.scalar.activation(
                                ugs[g][:, ft, :], pss[f4][:], AF.Gelu,
                                bias=colc[:, 144 + lw * 32 + ft:144 + lw * 32 + ft + 1],
                            )
                # ---- FFN down: both streams' psum groups held across q4 ----
                b2row = gbpool.tile([1, C], BF, tag="b2", name="b2row")
                nc.gpsimd.dma_start(out=b2row[:], in_=b21_d[lw:lw + 1, :])
                for nf in range(2):
                    pss2 = [[psA.tile([128, 512], F32, tag="ps", name="acc2")
                             for _ in range(2)] for g in range(2)]
                    for q4 in range(4):
                        w2t = load_wb(w2_d, lw * FF * C, q4 * 1024, nf * 512, 8, 512, C)
                        for g in range(2):
                            for jj in range(8):
                                ctf = q4 * 8 + jj
                                for tl in range(2):
                                    nc.tensor.matmul(
                                        pss2[g][tl][:],
                                        ugs[g][:, ctf, tl * 128:(tl + 1) * 128],
                                        w2t[:, jj, :],
                                        start=(ctf == 0), stop=False,
                                    )
                    for g in range(2):
                        for tl in range(2):
                            ps = pss2[g][tl]
                            nc.tensor.matmul(
                                ps[:], ones1[0:1, :], b2row[0:1, nf * 512:(nf + 1) * 512],
                                start=False, stop=True,
                            )
                            xs = x_t[2 * g + tl][:, nf * 512:(nf + 1) * 512]
                            nc.vector.tensor_add(out=xs, in0=xs, in1=ps[:])

                if debug:
                    for tt in range(NT):
                        nc.sync.dma_start(
                            out=dbg_d[l, tt * 128:(tt + 1) * 128, :], in_=x_t[tt][:]
                        )

            # ---- final LN + token-sharded lm_head over full (padded) vocab ----
            hfs = [emit_ln(g, 128, 136, "hfT")[0] for g in range(2)]

            for vc in range(NVC):
                wt = lmw.tile([128, 8, VCW], BF, tag="lmw", name="lmwt")
                nc.sync.dma_start(
                    out=wt[:],
                    in_=_ap(wlm_d, vc * VCW, [[VP, 128], [128 * VP, 8], [1, VCW]]),
                )
                lg = lgpool.tile([128, 4, VCW], FP16, tag="lg", name="lg")
                for ts in range(NT):
                    hf = hfs[ts // 2]
                    tl = ts % 2
                    ps = psA.tile([128, VCW], F32, tag="ps", name="lmacc")
                    for ct in range(NCT):
                        nc.tensor.matmul(
                            ps[:], hf[:, ct, tl * 128:(tl + 1) * 128],
                            wt[:, ct, :],
                            start=(ct == 0), stop=(ct == NCT - 1),
                        )
                    if ts % 2 == 0:
                        nc.scalar.activation(lg[:, ts, :], ps[:], AF.Copy)
                    else:
                        nc.vector.tensor_copy(out=lg[:, ts, :], in_=ps[:])
                nc.sync.dma_start(
                    out=_ap(logits_d, vc * VCW, [[VP, 128], [128 * VP, 4], [1, VCW]]),
                    in_=lg[:],
                )

    nc.compile()
    _prog_cache[key] = nc
    return nc


def _pack_colc(f):
    # [128, 272] f32, partition-major packed per-channel constants
    out = np.zeros((128, 272), dtype=np.float32)

    def cols(a):  # [L, C] -> [128, L*8]: out[p, l*8+ct] = a[l, ct*128+p]
        return a.reshape(L, NCT, 128).transpose(2, 0, 1).reshape(128, L * NCT)

    out[:, 0:32] = cols(f["ln1_g"].astype(np.float32))
    out[:, 32:64] = cols(f["ln1_b"].astype(np.float32))
    out[:, 64:96] = cols(f["ln2_g"].astype(np.float32))
    out[:, 96:128] = cols(f["ln2_b"].astype(np.float32))
    out[:, 128:136] = f["lnf_g"].astype(np.float32).reshape(NCT, 128).T
    out[:, 136:144] = f["lnf_b"].astype(np.float32).reshape(NCT, 128).T
    out[:, 144:272] = f["b1"].astype(np.float32).reshape(L, 32, 128).transpose(2, 0, 1).reshape(128, L * 32)
    return np.ascontiguousarray(out)


def _prep_inputs(inputs):
    f = {k: np.asarray(v) for k, v in inputs.items()}
    idx = f["idx"].astype(np.int64)
    emb = f["emb"].astype(np.float32)
    pos = f["pos_enc"].astype(np.float32)
    # residual stream carried at 256x (LN is affine-invariant; Wo/W2 are
    # pre-scaled by 256 so their PSUM deltas add in without a rescale pass)
    x_full = RS * (emb[idx] + pos[None, :, :])          # [B, T, C] f32

    fp8w = lambda a: np.ascontiguousarray(np.asarray(a, dtype=np.float32) * SW).astype(FP8NP)
    bfw = lambda a, s=1.0: np.ascontiguousarray(np.asarray(a, dtype=np.float32) * s).astype(BF16NP)
    shared = {
        "wq": fp8w(f["Wq"]),
        "wk": fp8w(f["Wk"]),
        "wv": bfw(f["Wv"]),
        "wo": bfw(f["Wo"], RS),
        "w1": bfw(f["W1"]),
        "w2": bfw(f["W2"], RS),
        "bo1": np.ascontiguousarray(f["bo"] * RS).astype(BF16NP),
        "b21": np.ascontiguousarray(f["b2"] * RS).astype(BF16NP),
        "colc": _pack_colc(f),
        "wlm": bfw(np.pad(f["Wlm"].astype(np.float32), ((0, 0), (0, VP - V)))),
        "tri": np.triu(np.ones((128, 128), dtype=np.float32)).astype(BF16NP),
    }

    in_maps = []
    for c in range(NCORES):
        x0_c = np.ascontiguousarray(
            x_full[:, 128 * c:128 * (c + 1), :].reshape(TL, C), dtype=np.float32
        )
        im = dict(shared)
        im["x0"] = x0_c
        in_maps.append(im)
    return in_maps


def kernel(**inputs):
    nc = _build()
    in_maps = _prep_inputs(inputs)
    res = run_bass_kernel_spmd(nc, in_maps, list(range(NCORES)))
    blm = np.asarray(inputs["blm"], dtype=np.float32)
    # core r holds rows (s*128 + j) = token 128r+j of seq s, full vocab
    full = np.empty((B, T, V), dtype=np.float32)
    for r in range(NCORES):
        lr = np.asarray(res.results[r]["logits"], dtype=np.float32).reshape(SEQ, 128, VP)
        full[:, 128 * r:128 * (r + 1), :] = lr[:, :, :V]
    full += blm[None, None, :]
    return full
